# revision 12
# baseline (speedup 1.0000x reference)
"""Trainium2 Bass kernel for nn_ConvAConnect (per-sample-weight 3x3 conv).

Strategy (pure data parallel, 16 samples per core on 8 cores):
  - The 3x3xCinxCout conv with per-sample weights is mapped to PE matmuls
    via a block-Toeplitz weight matrix per (sample, kh):
        lhsT  [K=128, M=112]: K = 16 input pixels x 8 cin of an x-strip,
                              M = 14 output pixels x 8 cout.
        rhs   [K=128, N=256]: the same x-strip of the (host-)transposed
                              input, streamed along y.
        out   [112, 256] PSUM, accumulated over the 3 kh taps (y-shifts
                              are free-dim offsets on the rhs).
  - Input is host-transposed to [(x*8+ci), y] layout (+ zero halos) so all
    device DMAs are wide contiguous runs; output is written in
    [(xo*8+co), y] strip layout and un-permuted on the host.
  - Bias (bias*Berr) is fused into the PSUM->SBUF copy on the scalar engine.
"""

import os
import sys

import numpy as np

for _p in ("/opt/trn_rl_repo", "/root/.axon_site/_ro/trn_rl_repo"):
    if os.path.isdir(_p) and _p not in sys.path:
        sys.path.insert(0, _p)

# Problem dims (hardcoded per spec)
B, H, W_IMG, CIN = 128, 256, 256, 8
KH, KW, COUT = 3, 3, 8

NCORES = 8
BPC = B // NCORES  # samples per core = 16
NPIX = 14          # output pixels per strip
SW = NPIX * COUT   # 112 psum partitions per strip
NS = -(-W_IMG // NPIX)  # 19 strips
KP = 128           # contraction: 16 in-pixels x 8 cin
Y = H              # 256
YP = Y + 2         # y-padded (halo col 0 and 257)
XROWS = 112 * (NS - 1) + KP + 16  # 2160 -> round up a bit
XROWS = 2176

TRACE = False       # test.py can flip this to profile
LAST_RESULT = [None]

_NC_CACHE = [None]


def _build_nc():
    import concourse.bass as bass
    import concourse.mybir as mybir
    from concourse.tile import TileContext

    f32 = mybir.dt.float32
    nc = bass.Bass()
    xs = nc.declare_dram_parameter("xs", [BPC, KP, NS, YP], f32, isOutput=False)
    tw = nc.declare_dram_parameter("tw", [BPC, KP, KH, SW], f32, isOutput=False)
    bi = nc.declare_dram_parameter("bi", [SW, BPC], f32, isOutput=False)
    zt = nc.declare_dram_parameter("zt", [BPC, SW, NS, Y], f32, isOutput=True)

    groups = [list(range(0, 8)), list(range(8, 16)), list(range(16, NS))]

    with TileContext(nc) as tc:
        with (
            tc.tile_pool(name="xp", bufs=3) as xp,
            tc.tile_pool(name="wp", bufs=2) as wp,
            tc.tile_pool(name="op", bufs=3) as op,
            tc.tile_pool(name="bp", bufs=1) as bp,
            tc.tile_pool(name="pp", bufs=8, space="PSUM") as pp,
        ):
            bias_t = bp.tile([SW, BPC], f32)
            nc.sync.dma_start(out=bias_t, in_=bi[:, :])
            for b in range(BPC):
                xtile = xp.tile([KP, NS, YP], f32)
                nc.sync.dma_start(out=xtile, in_=xs[b])
                wtile = wp.tile([KP, KH, SW], f32)
                nc.sync.dma_start(out=wtile, in_=tw[b])
                for grp in groups:
                    # one strip per PSUM bank tile: matmul start=True clears
                    # has_written for the WHOLE bank, so banks can't be shared
                    # between concurrently-open accumulation groups.
                    pts = [pp.tile([128, Y], f32, name="pt", tag="pt")
                           for _ in grp]
                    otile = op.tile([SW, 8 * Y], f32)
                    for kh in range(KH):
                        lhsT = wtile[:, kh, :]
                        for j, s in enumerate(grp):
                            nc.tensor.matmul(
                                out=pts[j][0:SW, 0:Y],
                                lhsT=lhsT,
                                rhs=xtile[:, s, kh:kh + Y],
                                start=(kh == 0),
                                stop=(kh == KH - 1),
                            )
                    for j in range(len(grp)):
                        nc.scalar.add(
                            out=otile[0:SW, j * Y:(j + 1) * Y],
                            in_=pts[j][0:SW, 0:Y],
                            add=bias_t[:, b:b + 1],
                        )
                    nc.sync.dma_start(
                        out=zt[b, :, grp[0]:grp[0] + len(grp), :],
                        in_=otile[0:SW, 0:len(grp) * Y],
                    )
    _split_multi_waits(nc, mybir)
    return nc


def _split_multi_waits(nc, mybir):
    """This walrus build supports only ONE sync-wait per compute-engine
    instruction (LDW/AC structs reject more). Tile attaches several at join
    points; hoist the extras onto injected same-engine NOPs just before."""
    nid = [0]
    for fn in nc.m.functions:
        for blk in fn.blocks:
            out = []
            for inst in blk.instructions:
                si = inst.sync_info
                if si is not None and si.on_wait and len(si.on_wait) > 1:
                    waits = list(si.on_wait)
                    for w in waits[:-1]:
                        nid[0] += 1
                        out.append(mybir.InstNoOp(
                            name=f"nopw-{nid[0]}",
                            engine=inst.engine,
                            ins=[],
                            outs=[],
                            sync_info=mybir.SyncInfo(on_wait=[w], on_update=[]),
                        ))
                    inst.sync_info = mybir.SyncInfo(
                        on_wait=[waits[-1]],
                        on_update=list(si.on_update or []),
                    )
                out.append(inst)
            blk.instructions[:] = out


def _get_nc():
    if _NC_CACHE[0] is None:
        _NC_CACHE[0] = _build_nc()
    return _NC_CACHE[0]


def host_prep(X, W, bias, Werr, Berr):
    """Host-side layout prep: per-sample Toeplitz weights, transposed input."""
    X = np.asarray(X, np.float32)
    W = np.asarray(W, np.float32)
    bias = np.asarray(bias, np.float32)
    Werr = np.asarray(Werr, np.float32)
    Berr = np.asarray(Berr, np.float32)

    memW = W[None] * Werr  # [B, kh, kw, ci, co]
    TW = np.zeros((B, KP, KH, SW), np.float32)
    for kw in range(KW):
        # [B, kh, ci, co] -> (b, ci, kh, co)
        blk = memW[:, :, kw].transpose(0, 2, 1, 3)
        for xo in range(NPIX):
            xi = xo + kw
            TW[:, xi * 8:(xi + 1) * 8, :, xo * 8:(xo + 1) * 8] = blk

    BIT = np.tile(bias[None] * Berr, (1, NPIX))  # [B, 112]

    XT = np.zeros((B, XROWS, Y), np.float32)
    XT[:, 8:8 + W_IMG * CIN, :] = X.transpose(0, 2, 3, 1).reshape(B, W_IMG * CIN, Y)
    XS = np.zeros((B, KP, NS, YP), np.float32)
    for s in range(NS):
        XS[:, :, s, 1:1 + Y] = XT[:, 112 * s:112 * s + KP, :]
    return XS, TW, BIT


def host_unpack(zt_all):
    """[B, 112, 19, 256] strip layout -> [B, H, W, COUT]."""
    z = zt_all.reshape(B, NPIX, COUT, NS, Y)
    z = z.transpose(0, 4, 3, 1, 2).reshape(B, Y, NS * NPIX, COUT)
    return np.ascontiguousarray(z[:, :, :W_IMG, :])


def kernel(X, W, bias, Werr, Berr):
    from concourse.bass_utils import run_bass_kernel_spmd

    XS, TW, BIT = host_prep(X, W, bias, Werr, Berr)
    in_maps = []
    for m in range(NCORES):
        sl = slice(m * BPC, (m + 1) * BPC)
        in_maps.append({
            "xs": np.ascontiguousarray(XS[sl]),
            "tw": np.ascontiguousarray(TW[sl]),
            "bi": np.ascontiguousarray(BIT[sl].T),
        })
    nc = _get_nc()
    res = run_bass_kernel_spmd(nc, in_maps, core_ids=list(range(NCORES)), trace=TRACE)
    LAST_RESULT[0] = res
    zt_all = np.concatenate([r["zt"] for r in res.results], axis=0)
    return host_unpack(zt_all)


# revision 14
# speedup vs baseline: 1.7295x; 1.7295x over previous
"""Trainium2 Bass kernel for nn_ConvAConnect (per-sample-weight 3x3 conv).

Strategy (pure data parallel, 16 samples per core on 8 cores):
  - The 3x3xCinxCout conv with per-sample weights is mapped to PE matmuls
    via a block-Toeplitz weight matrix per (sample, kh):
        lhsT  [K=128, M=112]: K = 16 input pixels x 8 cin of an x-strip,
                              M = 14 output pixels x 8 cout.
        rhs   [K=128, N=256]: the same x-strip of the (host-)transposed
                              input, streamed along y.
        out   [112, 256] PSUM, accumulated over the 3 kh taps (y-shifts
                              are free-dim offsets on the rhs).
  - Input is host-transposed to [(x*8+ci), y] layout (+ zero halos) so all
    device DMAs are wide contiguous runs; output is written in
    [(xo*8+co), y] strip layout and un-permuted on the host.
  - Bias (bias*Berr) is fused into the PSUM->SBUF copy on the scalar engine.
"""

import os
import sys

import numpy as np

for _p in ("/opt/trn_rl_repo", "/root/.axon_site/_ro/trn_rl_repo"):
    if os.path.isdir(_p) and _p not in sys.path:
        sys.path.insert(0, _p)

# Problem dims (hardcoded per spec)
B, H, W_IMG, CIN = 128, 256, 256, 8
KH, KW, COUT = 3, 3, 8

NCORES = 8
BPC = B // NCORES  # samples per core = 16
NPIX = 14          # output pixels per strip
SW = NPIX * COUT   # 112 psum partitions per strip
NS = -(-W_IMG // NPIX)  # 19 strips
KP = 128           # contraction: 16 in-pixels x 8 cin
Y = H              # 256
YP = Y + 2         # y-padded (halo col 0 and 257)
XROWS = 112 * (NS - 1) + KP + 16  # 2160 -> round up a bit
XROWS = 2176

TRACE = False       # test.py can flip this to profile
LAST_RESULT = [None]

_NC_CACHE = [None]


def _build_nc():
    import concourse.bass as bass
    import concourse.mybir as mybir
    from concourse.tile import TileContext

    f32 = mybir.dt.float32
    bf16 = mybir.dt.bfloat16
    nc = bass.Bass()
    xs = nc.declare_dram_parameter("xs", [BPC, KP, NS, YP], f32, isOutput=False)
    tw = nc.declare_dram_parameter("tw", [BPC, KP, KH, SW], f32, isOutput=False)
    bi = nc.declare_dram_parameter("bi", [SW, BPC], f32, isOutput=False)
    zt = nc.declare_dram_parameter("zt", [BPC, SW, NS, Y], f32, isOutput=True)

    groups = [list(range(0, 8)), list(range(8, 16)), list(range(16, NS))]

    with TileContext(nc) as tc:
        with (
            tc.tile_pool(name="xp", bufs=3) as xp,
            tc.tile_pool(name="wp", bufs=2) as wp,
            tc.tile_pool(name="op", bufs=3) as op,
            tc.tile_pool(name="bp", bufs=1) as bp,
            tc.tile_pool(name="pp", bufs=8, space="PSUM") as pp,
        ):
            bias_t = bp.tile([SW, BPC], f32)
            nc.sync.dma_start(out=bias_t, in_=bi[:, :])
            for b in range(BPC):
                # SWDGE (gpsimd) DMAs cast f32->bf16 in flight: HBM traffic
                # stays f32, SBUF gets bf16 for single-pass PE streaming.
                # Also puts input DMAs on a separate queue from the output
                # HWDGE ring (no head-of-line blocking between in and out).
                xtile = xp.tile([KP, NS, YP], bf16)
                nc.gpsimd.dma_start(out=xtile, in_=xs[b])
                wtile = wp.tile([KP, KH, SW], bf16)
                nc.gpsimd.dma_start(out=wtile, in_=tw[b])
                for grp in groups:
                    # one strip per PSUM bank tile: matmul start=True clears
                    # has_written for the WHOLE bank, so banks can't be shared
                    # between concurrently-open accumulation groups.
                    pts = [pp.tile([128, Y], f32, name="pt", tag="pt")
                           for _ in grp]
                    otile = op.tile([SW, 8 * Y], f32)
                    for kh in range(KH):
                        lhsT = wtile[:, kh, :]
                        for j, s in enumerate(grp):
                            nc.tensor.matmul(
                                out=pts[j][0:SW, 0:Y],
                                lhsT=lhsT,
                                rhs=xtile[:, s, kh:kh + Y],
                                start=(kh == 0),
                                stop=(kh == KH - 1),
                            )
                    for j in range(len(grp)):
                        nc.scalar.add(
                            out=otile[0:SW, j * Y:(j + 1) * Y],
                            in_=pts[j][0:SW, 0:Y],
                            add=bias_t[:, b:b + 1],
                        )
                    nc.sync.dma_start(
                        out=zt[b, :, grp[0]:grp[0] + len(grp), :],
                        in_=otile[0:SW, 0:len(grp) * Y],
                    )
    _split_multi_waits(nc, mybir)
    return nc


def _split_multi_waits(nc, mybir):
    """This walrus build supports only ONE sync-wait per compute-engine
    instruction (LDW/AC structs reject more). Tile attaches several at join
    points; hoist the extras onto injected same-engine NOPs just before."""
    nid = [0]
    for fn in nc.m.functions:
        for blk in fn.blocks:
            out = []
            for inst in blk.instructions:
                si = inst.sync_info
                if si is not None and si.on_wait and len(si.on_wait) > 1:
                    waits = list(si.on_wait)
                    for w in waits[:-1]:
                        nid[0] += 1
                        out.append(mybir.InstNoOp(
                            name=f"nopw-{nid[0]}",
                            engine=inst.engine,
                            ins=[],
                            outs=[],
                            sync_info=mybir.SyncInfo(on_wait=[w], on_update=[]),
                        ))
                    inst.sync_info = mybir.SyncInfo(
                        on_wait=[waits[-1]],
                        on_update=list(si.on_update or []),
                    )
                out.append(inst)
            blk.instructions[:] = out


def _get_nc():
    if _NC_CACHE[0] is None:
        _NC_CACHE[0] = _build_nc()
    return _NC_CACHE[0]


def host_prep(X, W, bias, Werr, Berr):
    """Host-side layout prep: per-sample Toeplitz weights, transposed input."""
    X = np.asarray(X, np.float32)
    W = np.asarray(W, np.float32)
    bias = np.asarray(bias, np.float32)
    Werr = np.asarray(Werr, np.float32)
    Berr = np.asarray(Berr, np.float32)

    memW = W[None] * Werr  # [B, kh, kw, ci, co]
    TW = np.zeros((B, KP, KH, SW), np.float32)
    for kw in range(KW):
        # [B, kh, ci, co] -> (b, ci, kh, co)
        blk = memW[:, :, kw].transpose(0, 2, 1, 3)
        for xo in range(NPIX):
            xi = xo + kw
            TW[:, xi * 8:(xi + 1) * 8, :, xo * 8:(xo + 1) * 8] = blk

    BIT = np.tile(bias[None] * Berr, (1, NPIX))  # [B, 112]

    XT = np.zeros((B, XROWS, Y), np.float32)
    XT[:, 8:8 + W_IMG * CIN, :] = X.transpose(0, 2, 3, 1).reshape(B, W_IMG * CIN, Y)
    XS = np.zeros((B, KP, NS, YP), np.float32)
    for s in range(NS):
        XS[:, :, s, 1:1 + Y] = XT[:, 112 * s:112 * s + KP, :]
    return XS, TW, BIT


def host_unpack(zt_all):
    """[B, 112, 19, 256] strip layout -> [B, H, W, COUT]."""
    z = zt_all.reshape(B, NPIX, COUT, NS, Y)
    z = z.transpose(0, 4, 3, 1, 2).reshape(B, Y, NS * NPIX, COUT)
    return np.ascontiguousarray(z[:, :, :W_IMG, :])


def kernel(X, W, bias, Werr, Berr):
    from concourse.bass_utils import run_bass_kernel_spmd

    XS, TW, BIT = host_prep(X, W, bias, Werr, Berr)
    in_maps = []
    for m in range(NCORES):
        sl = slice(m * BPC, (m + 1) * BPC)
        in_maps.append({
            "xs": np.ascontiguousarray(XS[sl]),
            "tw": np.ascontiguousarray(TW[sl]),
            "bi": np.ascontiguousarray(BIT[sl].T),
        })
    nc = _get_nc()
    res = run_bass_kernel_spmd(nc, in_maps, core_ids=list(range(NCORES)), trace=TRACE)
    LAST_RESULT[0] = res
    zt_all = np.concatenate([r["zt"] for r in res.results], axis=0)
    return host_unpack(zt_all)


# revision 17
# speedup vs baseline: 2.0885x; 1.2076x over previous
"""Trainium2 Bass kernel for nn_ConvAConnect (per-sample-weight 3x3 conv).

Strategy (pure data parallel, 16 samples per core on 8 cores):
  - The 3x3xCinxCout conv with per-sample weights is mapped to PE matmuls
    via a block-Toeplitz weight matrix per (sample, kh):
        lhsT  [K=128, M=112]: K = 16 input pixels x 8 cin of an x-strip,
                              M = 14 output pixels x 8 cout.
        rhs   [K=128, N=256]: the same x-strip of the (host-)transposed
                              input, streamed along y.
        out   [112, 256] PSUM, accumulated over the 3 kh taps (y-shifts
                              are free-dim offsets on the rhs).
  - Input is host-transposed to [(x*8+ci), y] layout (+ zero halos) so all
    device DMAs are wide contiguous runs; output is written in
    [(xo*8+co), y] strip layout and un-permuted on the host.
  - Bias (bias*Berr) is fused into the PSUM->SBUF copy on the scalar engine.
"""

import os
import sys

import numpy as np

for _p in ("/opt/trn_rl_repo", "/root/.axon_site/_ro/trn_rl_repo"):
    if os.path.isdir(_p) and _p not in sys.path:
        sys.path.insert(0, _p)

# Problem dims (hardcoded per spec)
B, H, W_IMG, CIN = 128, 256, 256, 8
KH, KW, COUT = 3, 3, 8

NCORES = 8
BPC = B // NCORES  # samples per core = 16
NPIX = 14          # output pixels per strip
SW = NPIX * COUT   # 112 psum partitions per strip
NS = -(-W_IMG // NPIX)  # 19 strips
KP = 128           # contraction: 16 in-pixels x 8 cin
Y = H              # 256
YP = Y + 2         # y-padded (halo col 0 and 257)
XROWS = 112 * (NS - 1) + KP + 16  # 2160 -> round up a bit
XROWS = 2176

TRACE = False       # test.py can flip this to profile
LAST_RESULT = [None]

_NC_CACHE = [None]


def _build_nc():
    import concourse.bass as bass
    import concourse.mybir as mybir
    from concourse.tile import TileContext

    f32 = mybir.dt.float32
    bf16 = mybir.dt.bfloat16
    nc = bass.Bass()
    xs = nc.declare_dram_parameter("xs", [BPC, KP, NS, YP], bf16, isOutput=False)
    tw = nc.declare_dram_parameter("tw", [BPC, KP, KH, SW], bf16, isOutput=False)
    bi = nc.declare_dram_parameter("bi", [SW, BPC], f32, isOutput=False)
    zt = nc.declare_dram_parameter("zt", [BPC, SW, NS, Y], f32, isOutput=True)

    groups = [list(range(0, 8)), list(range(8, 16)), list(range(16, NS))]

    with TileContext(nc) as tc:
        with (
            tc.tile_pool(name="xp", bufs=3) as xp,
            tc.tile_pool(name="wp", bufs=2) as wp,
            tc.tile_pool(name="op", bufs=3) as op,
            tc.tile_pool(name="bp", bufs=1) as bp,
            tc.tile_pool(name="pp", bufs=8, space="PSUM") as pp,
        ):
            bias_t = bp.tile([SW, BPC], f32)
            nc.sync.dma_start(out=bias_t, in_=bi[:, :])
            for b in range(BPC):
                # Inputs ship as bf16 (host-cast): half the read traffic, and
                # single-pass PE streaming. gpsimd (SWDGE) queue keeps input
                # DMAs off the output HWDGE ring (no head-of-line blocking).
                xtile = xp.tile([KP, NS, YP], bf16)
                nc.gpsimd.dma_start(out=xtile, in_=xs[b])
                wtile = wp.tile([KP, KH, SW], bf16)
                nc.gpsimd.dma_start(out=wtile, in_=tw[b])
                for grp in groups:
                    # one strip per PSUM bank tile: matmul start=True clears
                    # has_written for the WHOLE bank, so banks can't be shared
                    # between concurrently-open accumulation groups.
                    pts = [pp.tile([128, Y], f32, name="pt", tag="pt")
                           for _ in grp]
                    otile = op.tile([SW, 8 * Y], f32)
                    for kh in range(KH):
                        lhsT = wtile[:, kh, :]
                        for j, s in enumerate(grp):
                            nc.tensor.matmul(
                                out=pts[j][0:SW, 0:Y],
                                lhsT=lhsT,
                                rhs=xtile[:, s, kh:kh + Y],
                                start=(kh == 0),
                                stop=(kh == KH - 1),
                            )
                    for j in range(len(grp)):
                        nc.scalar.add(
                            out=otile[0:SW, j * Y:(j + 1) * Y],
                            in_=pts[j][0:SW, 0:Y],
                            add=bias_t[:, b:b + 1],
                        )
                    nc.sync.dma_start(
                        out=zt[b, :, grp[0]:grp[0] + len(grp), :],
                        in_=otile[0:SW, 0:len(grp) * Y],
                    )
    _split_multi_waits(nc, mybir)
    return nc


def _split_multi_waits(nc, mybir):
    """This walrus build supports only ONE sync-wait per compute-engine
    instruction (LDW/AC structs reject more). Tile attaches several at join
    points; hoist the extras onto injected same-engine NOPs just before."""
    nid = [0]
    for fn in nc.m.functions:
        for blk in fn.blocks:
            out = []
            for inst in blk.instructions:
                si = inst.sync_info
                if si is not None and si.on_wait and len(si.on_wait) > 1:
                    waits = list(si.on_wait)
                    for w in waits[:-1]:
                        nid[0] += 1
                        out.append(mybir.InstNoOp(
                            name=f"nopw-{nid[0]}",
                            engine=inst.engine,
                            ins=[],
                            outs=[],
                            sync_info=mybir.SyncInfo(on_wait=[w], on_update=[]),
                        ))
                    inst.sync_info = mybir.SyncInfo(
                        on_wait=[waits[-1]],
                        on_update=list(si.on_update or []),
                    )
                out.append(inst)
            blk.instructions[:] = out


def _get_nc():
    if _NC_CACHE[0] is None:
        _NC_CACHE[0] = _build_nc()
    return _NC_CACHE[0]


def host_prep(X, W, bias, Werr, Berr):
    """Host-side layout prep: per-sample Toeplitz weights, transposed input."""
    X = np.asarray(X, np.float32)
    W = np.asarray(W, np.float32)
    bias = np.asarray(bias, np.float32)
    Werr = np.asarray(Werr, np.float32)
    Berr = np.asarray(Berr, np.float32)

    memW = W[None] * Werr  # [B, kh, kw, ci, co]
    TW = np.zeros((B, KP, KH, SW), np.float32)
    for kw in range(KW):
        # [B, kh, ci, co] -> (b, ci, kh, co)
        blk = memW[:, :, kw].transpose(0, 2, 1, 3)
        for xo in range(NPIX):
            xi = xo + kw
            TW[:, xi * 8:(xi + 1) * 8, :, xo * 8:(xo + 1) * 8] = blk

    BIT = np.tile(bias[None] * Berr, (1, NPIX))  # [B, 112]

    import ml_dtypes
    bf16 = ml_dtypes.bfloat16
    XT = np.zeros((B, XROWS, Y), bf16)
    XT[:, 8:8 + W_IMG * CIN, :] = X.transpose(0, 2, 3, 1).reshape(B, W_IMG * CIN, Y)
    XS = np.zeros((B, KP, NS, YP), bf16)
    for s in range(NS):
        XS[:, :, s, 1:1 + Y] = XT[:, 112 * s:112 * s + KP, :]
    return XS, TW.astype(bf16), BIT


def host_unpack(zt_all):
    """[B, 112, 19, 256] strip layout -> [B, H, W, COUT]."""
    z = zt_all.reshape(B, NPIX, COUT, NS, Y)
    z = z.transpose(0, 4, 3, 1, 2).reshape(B, Y, NS * NPIX, COUT)
    return np.ascontiguousarray(z[:, :, :W_IMG, :])


def kernel(X, W, bias, Werr, Berr):
    from concourse.bass_utils import run_bass_kernel_spmd

    XS, TW, BIT = host_prep(X, W, bias, Werr, Berr)
    in_maps = []
    for m in range(NCORES):
        sl = slice(m * BPC, (m + 1) * BPC)
        in_maps.append({
            "xs": np.ascontiguousarray(XS[sl]),
            "tw": np.ascontiguousarray(TW[sl]),
            "bi": np.ascontiguousarray(BIT[sl].T),
        })
    nc = _get_nc()
    res = run_bass_kernel_spmd(nc, in_maps, core_ids=list(range(NCORES)), trace=TRACE)
    LAST_RESULT[0] = res
    zt_all = np.concatenate([r["zt"] for r in res.results], axis=0)
    return host_unpack(zt_all)


# revision 19
# speedup vs baseline: 2.4341x; 1.1655x over previous
"""Trainium2 Bass kernel for nn_ConvAConnect (per-sample-weight 3x3 conv).

Strategy (pure data parallel, 16 samples per core on 8 cores):
  - The 3x3xCinxCout conv with per-sample weights is mapped to PE matmuls
    via a block-Toeplitz weight matrix per (sample, kh):
        lhsT  [K=128, M=112]: K = 16 input pixels x 8 cin of an x-strip,
                              M = 14 output pixels x 8 cout.
        rhs   [K=128, N=256]: the same x-strip of the (host-)transposed
                              input, streamed along y.
        out   [112, 256] PSUM, accumulated over the 3 kh taps (y-shifts
                              are free-dim offsets on the rhs).
  - Input is host-transposed to [(x*8+ci), y] layout (+ zero halos) so all
    device DMAs are wide contiguous runs; output is written in
    [(xo*8+co), y] strip layout and un-permuted on the host.
  - Bias (bias*Berr) is fused into the PSUM->SBUF copy on the scalar engine.
"""

import os
import sys

import numpy as np

for _p in ("/opt/trn_rl_repo", "/root/.axon_site/_ro/trn_rl_repo"):
    if os.path.isdir(_p) and _p not in sys.path:
        sys.path.insert(0, _p)

# Problem dims (hardcoded per spec)
B, H, W_IMG, CIN = 128, 256, 256, 8
KH, KW, COUT = 3, 3, 8

NCORES = 8
BPC = B // NCORES  # samples per core = 16
NPIX = 14          # output pixels per strip
SW = NPIX * COUT   # 112 psum partitions per strip
NS = -(-W_IMG // NPIX)  # 19 strips
KP = 128           # contraction: 16 in-pixels x 8 cin
Y = H              # 256
YP = Y + 2         # y-padded (halo col 0 and 257)
XROWS = 112 * (NS - 1) + KP + 16  # 2160 -> round up a bit
XROWS = 2176

TRACE = False       # test.py can flip this to profile
LAST_RESULT = [None]

_NC_CACHE = [None]


def _build_nc():
    import concourse.bass as bass
    import concourse.mybir as mybir
    from concourse.tile import TileContext

    f32 = mybir.dt.float32
    bf16 = mybir.dt.bfloat16
    nc = bass.Bass()
    xs = nc.declare_dram_parameter("xs", [BPC, KP, NS, YP], bf16, isOutput=False)
    tw = nc.declare_dram_parameter("tw", [BPC, KP, KH, SW], bf16, isOutput=False)
    bi = nc.declare_dram_parameter("bi", [SW, BPC], f32, isOutput=False)
    zt = nc.declare_dram_parameter("zt", [BPC, SW, NS, Y], f32, isOutput=True)

    groups = [list(range(0, 8)), list(range(8, 16)), list(range(16, NS))]

    with TileContext(nc) as tc:
        with (
            tc.tile_pool(name="xp", bufs=4) as xp,
            tc.tile_pool(name="wp", bufs=3) as wp,
            tc.tile_pool(name="op", bufs=4) as op,
            tc.tile_pool(name="bp", bufs=1) as bp,
            tc.tile_pool(name="pp", bufs=8, space="PSUM") as pp,
        ):
            bias_t = bp.tile([SW, BPC], f32)
            nc.sync.dma_start(out=bias_t, in_=bi[:, :])
            for b in range(BPC):
                # Inputs ship as bf16 (host-cast): half the read traffic, and
                # single-pass PE streaming. gpsimd (SWDGE) queue keeps input
                # DMAs off the output HWDGE ring (no head-of-line blocking).
                xtile = xp.tile([KP, NS, YP], bf16)
                nc.gpsimd.dma_start(out=xtile, in_=xs[b])
                wtile = wp.tile([KP, KH, SW], bf16)
                nc.gpsimd.dma_start(out=wtile, in_=tw[b])
                for grp in groups:
                    # one strip per PSUM bank tile: matmul start=True clears
                    # has_written for the WHOLE bank, so banks can't be shared
                    # between concurrently-open accumulation groups.
                    pts = [pp.tile([128, Y], f32, name="pt", tag="pt")
                           for _ in grp]
                    otile = op.tile([SW, 8 * Y], f32)
                    for kh in range(KH):
                        lhsT = wtile[:, kh, :]
                        for j, s in enumerate(grp):
                            nc.tensor.matmul(
                                out=pts[j][0:SW, 0:Y],
                                lhsT=lhsT,
                                rhs=xtile[:, s, kh:kh + Y],
                                start=(kh == 0),
                                stop=(kh == KH - 1),
                            )
                    for j in range(len(grp)):
                        # split PSUM->SBUF (+bias) copies across ACT and DVE
                        if j % 2 == 0:
                            nc.scalar.add(
                                out=otile[0:SW, j * Y:(j + 1) * Y],
                                in_=pts[j][0:SW, 0:Y],
                                add=bias_t[:, b:b + 1],
                            )
                        else:
                            nc.vector.tensor_scalar_add(
                                out=otile[0:SW, j * Y:(j + 1) * Y],
                                in0=pts[j][0:SW, 0:Y],
                                scalar1=bias_t[:, b:b + 1],
                            )
                    nc.sync.dma_start(
                        out=zt[b, :, grp[0]:grp[0] + len(grp), :],
                        in_=otile[0:SW, 0:len(grp) * Y],
                    )
    _split_multi_waits(nc, mybir)
    return nc


def _split_multi_waits(nc, mybir):
    """This walrus build supports only ONE sync-wait per compute-engine
    instruction (LDW/AC structs reject more). Tile attaches several at join
    points; hoist the extras onto injected same-engine NOPs just before."""
    nid = [0]
    for fn in nc.m.functions:
        for blk in fn.blocks:
            out = []
            for inst in blk.instructions:
                si = inst.sync_info
                if si is not None and si.on_wait and len(si.on_wait) > 1:
                    waits = list(si.on_wait)
                    for w in waits[:-1]:
                        nid[0] += 1
                        out.append(mybir.InstNoOp(
                            name=f"nopw-{nid[0]}",
                            engine=inst.engine,
                            ins=[],
                            outs=[],
                            sync_info=mybir.SyncInfo(on_wait=[w], on_update=[]),
                        ))
                    inst.sync_info = mybir.SyncInfo(
                        on_wait=[waits[-1]],
                        on_update=list(si.on_update or []),
                    )
                out.append(inst)
            blk.instructions[:] = out


def _get_nc():
    if _NC_CACHE[0] is None:
        _NC_CACHE[0] = _build_nc()
    return _NC_CACHE[0]


def host_prep(X, W, bias, Werr, Berr):
    """Host-side layout prep: per-sample Toeplitz weights, transposed input."""
    X = np.asarray(X, np.float32)
    W = np.asarray(W, np.float32)
    bias = np.asarray(bias, np.float32)
    Werr = np.asarray(Werr, np.float32)
    Berr = np.asarray(Berr, np.float32)

    memW = W[None] * Werr  # [B, kh, kw, ci, co]
    TW = np.zeros((B, KP, KH, SW), np.float32)
    for kw in range(KW):
        # [B, kh, ci, co] -> (b, ci, kh, co)
        blk = memW[:, :, kw].transpose(0, 2, 1, 3)
        for xo in range(NPIX):
            xi = xo + kw
            TW[:, xi * 8:(xi + 1) * 8, :, xo * 8:(xo + 1) * 8] = blk

    BIT = np.tile(bias[None] * Berr, (1, NPIX))  # [B, 112]

    import ml_dtypes
    bf16 = ml_dtypes.bfloat16
    XT = np.zeros((B, XROWS, Y), bf16)
    XT[:, 8:8 + W_IMG * CIN, :] = X.transpose(0, 2, 3, 1).reshape(B, W_IMG * CIN, Y)
    XS = np.zeros((B, KP, NS, YP), bf16)
    for s in range(NS):
        XS[:, :, s, 1:1 + Y] = XT[:, 112 * s:112 * s + KP, :]
    return XS, TW.astype(bf16), BIT


def host_unpack(zt_all):
    """[B, 112, 19, 256] strip layout -> [B, H, W, COUT]."""
    z = zt_all.reshape(B, NPIX, COUT, NS, Y)
    z = z.transpose(0, 4, 3, 1, 2).reshape(B, Y, NS * NPIX, COUT)
    return np.ascontiguousarray(z[:, :, :W_IMG, :])


def kernel(X, W, bias, Werr, Berr):
    from concourse.bass_utils import run_bass_kernel_spmd

    XS, TW, BIT = host_prep(X, W, bias, Werr, Berr)
    in_maps = []
    for m in range(NCORES):
        sl = slice(m * BPC, (m + 1) * BPC)
        in_maps.append({
            "xs": np.ascontiguousarray(XS[sl]),
            "tw": np.ascontiguousarray(TW[sl]),
            "bi": np.ascontiguousarray(BIT[sl].T),
        })
    nc = _get_nc()
    res = run_bass_kernel_spmd(nc, in_maps, core_ids=list(range(NCORES)), trace=TRACE)
    LAST_RESULT[0] = res
    zt_all = np.concatenate([r["zt"] for r in res.results], axis=0)
    return host_unpack(zt_all)
